# revision 18
# baseline (speedup 1.0000x reference)
"""Multi-head attention (16 heads, S=4096, D=1024) on 8 TRN2 NeuronCores.

Megatron-style tensor parallelism over heads: core i owns heads (2i, 2i+1).
Each core computes its head slice of the q/k/v projections, full attention
for its 2 heads (writing the softmax probabilities, which are part of the
module output), and a rank-128 partial of the output projection. The host
sums the 8 partials (the "all-reduce") and concatenates the attention
probability slices.

Device algorithm per core (all matmuls in float32r):
  qhT[dh,s]  = (wq_slice @ q^T)            (dh = 128 = 2 heads x 64)
  khT, vhT   likewise; vh_aug[sk,65]       = [vh | 1] per head (PE transpose)
  pass A (per head h, sq-block j of 512):
      for sk-tile t: LT = khT_t^T-dot-qhT_j   -> exp(LT/tau)          (ACT)
                     ctxT[65,512] += [vh|1]^T @ exp  (PSUM accumulate)
      row 64 of ctxT = rowsum of exp over sk  -> rowsumT[h, j-block]
  pass B (per head h, sq-tile jj of 128):
      logits = qhT_jj^T-dot-khT  -> attn = exp(logits/tau - ln rowsum) (ACT)
      -> DMA to attn_out (normalized probabilities)
  ctxT normalized by 1/rowsum (PE broadcast + DVE mul), then
  partial[s,:] = ctxT_s^T @ woT_slice.
"""

import sys

sys.path.insert(0, "/opt/trn_rl_repo")

import numpy as np

import concourse.mybir as mybir
from concourse import bacc
from concourse.masks import make_identity
from concourse.tile import TileContext

F32 = mybir.dt.float32
F32R = mybir.dt.float32r
AF = mybir.ActivationFunctionType

D_MODEL = 1024
NUM_HEADS = 16
DEPTH = 64
TAU = 8.0
SEQ = 4096
N_CORES = 8
HPC = NUM_HEADS // N_CORES  # heads per core = 2
DH_SLICE = HPC * DEPTH  # 128 output dims per core


def build_mha_core(seq: int = SEQ, d_model: int = D_MODEL):
    """Build the per-core Bass module (same SPMD program on all 8 cores)."""
    S, D = seq, d_model
    KB = D // 128          # contraction blocks for projections
    NJ = S // 512          # sq blocks (pass A)
    NT = S // 128          # sk tiles (pass A) == sq tiles (pass B)
    NM = S // 512          # sk blocks (pass B)
    SCALE = 1.0 / TAU

    nc = bacc.Bacc("TRN2", target_bir_lowering=False)

    qT = nc.dram_tensor("qT", [D, S], F32R, kind="ExternalInput")
    kT = nc.dram_tensor("kT", [D, S], F32R, kind="ExternalInput")
    vT = nc.dram_tensor("vT", [D, S], F32R, kind="ExternalInput")
    wqT = nc.dram_tensor("wqT", [D, 128], F32R, kind="ExternalInput")
    wkT = nc.dram_tensor("wkT", [D, 128], F32R, kind="ExternalInput")
    wvT = nc.dram_tensor("wvT", [D, 128], F32R, kind="ExternalInput")
    bq = nc.dram_tensor("bq", [128, 1], F32, kind="ExternalInput")
    bk = nc.dram_tensor("bk", [128, 1], F32, kind="ExternalInput")
    bv = nc.dram_tensor("bv", [128, 1], F32, kind="ExternalInput")
    woT = nc.dram_tensor("woT", [128, D], F32R, kind="ExternalInput")
    attn_out = nc.dram_tensor("attn_out", [HPC, S, S], F32, kind="ExternalOutput")
    partial = nc.dram_tensor("partial", [S, D], F32, kind="ExternalOutput")

    with TileContext(nc) as tc:
        with (
            tc.tile_pool(name="consts", bufs=1) as consts,
            tc.tile_pool(name="persist", bufs=1) as persist,
            tc.tile_pool(name="xin", bufs=4) as xin,
            tc.tile_pool(name="expp", bufs=4) as expp,
            tc.tile_pool(name="attnb", bufs=2) as attnb,
            tc.tile_pool(name="outb", bufs=2) as outb,
            tc.tile_pool(name="lnp", bufs=2) as lnp,
            tc.tile_pool(name="ps_lt", bufs=2, space="PSUM") as ps_lt,
            tc.tile_pool(name="ps_ctx", bufs=2, space="PSUM") as ps_ctx,
            tc.tile_pool(name="ps_mm", bufs=3, space="PSUM") as ps_mm,
        ):
            # ---- constants -------------------------------------------------
            ident = consts.tile([128, 128], F32)
            make_identity(nc, ident)
            # row 64 of ones: stationary operand for broadcasting a value
            # sitting on partition 64 to 64 output partitions.
            ones_row = consts.tile([65, DEPTH], F32)
            nc.vector.memset(ones_row[64:65, :], 1.0)


            w_sb = {}
            b_sb = {}
            for name, wdram, bdram in (
                ("q", wqT, bq),
                ("k", wkT, bk),
                ("v", wvT, bv),
            ):
                w = consts.tile([128, D], F32R, tag=f"w{name}")
                for kb in range(KB):
                    nc.sync.dma_start(
                        out=w[:, kb * 128 : (kb + 1) * 128],
                        in_=wdram[kb * 128 : (kb + 1) * 128, :],
                    )
                w_sb[name] = w
                b = consts.tile([128, 1], F32, tag=f"b{name}")
                nc.sync.dma_start(out=b, in_=bdram[:, :])
                b_sb[name] = b
            # woT rows for head h at partition base 0: [64, HPC, D]
            woT_sb = consts.tile([64, HPC, D], F32R)
            for h in range(HPC):
                nc.sync.dma_start(
                    out=woT_sb[:, h, :], in_=woT[h * DEPTH : (h + 1) * DEPTH, :]
                )

            # ---- persistent tensors ---------------------------------------
            qhT_sb = persist.tile([128, S], F32R)
            khT_sb = persist.tile([128, S], F32R)
            vhT_sb = persist.tile([128, S], F32)
            vh_aug = persist.tile([128, HPC, NT, DEPTH + 1], F32R)
            # per-head ctx^T (rows 0..63) with the exp-rowsum in row 64
            ctxT_h = [
                persist.tile([65, S], F32R, tag=f"ctxT{h}", name=f"ctxT{h}")
                for h in range(HPC)
            ]
            nlog_cols = persist.tile([128, HPC, NT], F32)

            # ---- phase 1: projections -> qhT/khT/vhT [128, S] -------------
            for name, xdram, dest in (
                ("q", qT, qhT_sb),
                ("k", kT, khT_sb),
                ("v", vT, vhT_sb),
            ):
                w = w_sb[name]
                for n in range(S // 512):
                    ps = ps_mm.tile([128, 512], F32, tag="mm512")
                    for kb in range(KB):
                        xt = xin.tile([128, 512], F32R, tag="xin")
                        nc.sync.dma_start(
                            out=xt,
                            in_=xdram[
                                kb * 128 : (kb + 1) * 128, n * 512 : (n + 1) * 512
                            ],
                        )
                        nc.tensor.matmul(
                            ps,
                            lhsT=w[:, kb * 128 : (kb + 1) * 128],
                            rhs=xt,
                            start=(kb == 0),
                            stop=(kb == KB - 1),
                        )
                    # copy psum -> sbuf with per-partition bias add
                    nc.vector.tensor_scalar_add(
                        dest[:, n * 512 : (n + 1) * 512], ps, b_sb[name]
                    )

            # ---- phase 1.5: vh_aug = [vh | 1] per head --------------------
            nc.vector.memset(vh_aug[:, :, :, DEPTH : DEPTH + 1].bitcast(F32), 1.0)
            for h in range(HPC):
                hs = slice(h * DEPTH, (h + 1) * DEPTH)
                for t in range(NT):
                    pst = ps_mm.tile([128, DEPTH], F32, tag="mm512")
                    nc.tensor.transpose(
                        pst,
                        vhT_sb[hs, t * 128 : (t + 1) * 128],
                        ident[hs, hs],
                    )
                    nc.vector.tensor_copy(vh_aug[:, h, t, 0:DEPTH], pst)

            # ---- phase 2: attention ---------------------------------------
            for j in range(NJ):
                js = slice(j * 512, (j + 1) * 512)
                # pass A for both heads: ctxT accumulation + rowsums
                for h in range(HPC):
                    hs = slice(h * DEPTH, (h + 1) * DEPTH)
                    psc = ps_ctx.tile([DEPTH + 1, 512], F32, tag="ctx")
                    for t in range(NT):
                        psl = ps_lt.tile([128, 512], F32, tag="lt")
                        nc.tensor.matmul(
                            psl,
                            lhsT=khT_sb[hs, t * 128 : (t + 1) * 128],
                            rhs=qhT_sb[hs, js],
                            start=True,
                            stop=True,
                        )
                        ex = expp.tile([128, 512], F32R, tag="exp")
                        nc.scalar.activation(ex, psl, AF.Exp, scale=SCALE)
                        nc.tensor.matmul(
                            psc,
                            lhsT=vh_aug[:, h, t, :],
                            rhs=ex,
                            start=(t == 0),
                            stop=(t == NT - 1),
                        )
                    nc.vector.tensor_copy(ctxT_h[h][:, js], psc)
                    # -ln(rowsum) on partition 64, then transpose into
                    # per-partition bias columns for pass B
                    nlt = lnp.tile([65, 512], F32, tag="ln")
                    nc.scalar.activation(
                        nlt[64:65, :], psc[DEPTH : DEPTH + 1, :], AF.Ln
                    )
                    nc.vector.tensor_scalar_mul(nlt[64:65, :], nlt[64:65, :], -1.0)
                    for c in range(4):
                        pst = ps_mm.tile([128, 1], F32, tag="mm512")
                        nc.tensor.transpose(
                            pst,
                            nlt[64:65, c * 128 : (c + 1) * 128],
                            ident[64:65, 64:65],
                        )
                        nc.vector.tensor_copy(
                            nlog_cols[:, h, j * 4 + c : j * 4 + c + 1], pst
                        )
                # pass B: normalized probabilities -> DRAM
                for h in range(HPC):
                    hs = slice(h * DEPTH, (h + 1) * DEPTH)
                    for c in range(4):
                        jj = j * 4 + c
                        ab = attnb.tile([128, S], F32, tag="attn")
                        for m in range(NM):
                            pslg = ps_mm.tile([128, 512], F32, tag="mm512")
                            nc.tensor.matmul(
                                pslg,
                                lhsT=qhT_sb[hs, jj * 128 : (jj + 1) * 128].bitcast(
                                    F32R
                                ),
                                rhs=khT_sb[hs, m * 512 : (m + 1) * 512],
                                start=True,
                                stop=True,
                            )
                            nc.scalar.activation(
                                ab[:, m * 512 : (m + 1) * 512],
                                pslg,
                                AF.Exp,
                                scale=SCALE,
                                bias=nlog_cols[:, h, jj : jj + 1],
                            )
                        nc.sync.dma_start(
                            out=attn_out[h, jj * 128 : (jj + 1) * 128, :], in_=ab
                        )

            # ---- phase 3: normalize ctxT per head, output projection ------
            for h in range(HPC):
                ct = ctxT_h[h]
                for n in range(S // 512):
                    ns = slice(n * 512, (n + 1) * 512)
                    rn = lnp.tile([65, 512], F32, tag="rn")
                    nc.vector.reciprocal(rn[64:65, :], ct[64:65, ns].bitcast(F32))
                    psb = ps_mm.tile([DEPTH, 512], F32, tag="mm512")
                    # broadcast 1/rowsum (partition 64) to 64 partitions
                    nc.tensor.matmul(
                        psb,
                        lhsT=ones_row[64:65, :],
                        rhs=rn[64:65, :],
                        start=True,
                        stop=True,
                    )
                    nc.vector.tensor_mul(ct[0:DEPTH, ns], ct[0:DEPTH, ns], psb.bitcast(F32R))
            for st in range(NT):
                ob = outb.tile([128, D], F32, tag="ob")
                for dhf in range(D // 512):
                    pso = ps_mm.tile([128, 512], F32, tag="mm512")
                    for h in range(HPC):
                        nc.tensor.matmul(
                            pso,
                            lhsT=ctxT_h[h][0:DEPTH, st * 128 : (st + 1) * 128].bitcast(
                                F32R
                            ),
                            rhs=woT_sb[:, h, dhf * 512 : (dhf + 1) * 512].bitcast(
                                F32R
                            ),
                            start=(h == 0),
                            stop=(h == HPC - 1),
                        )
                    nc.vector.tensor_copy(ob[:, dhf * 512 : (dhf + 1) * 512], pso)
                nc.sync.dma_start(out=partial[st * 128 : (st + 1) * 128, :], in_=ob)

    nc.compile()
    return nc


def make_in_maps(q, k, v, wq_w, wq_b, wk_w, wk_b, wv_w, wv_b, wo_w):
    qT = np.ascontiguousarray(q.T)
    kT = np.ascontiguousarray(k.T)
    vT = np.ascontiguousarray(v.T)
    in_maps = []
    for i in range(N_CORES):
        sl = slice(i * DH_SLICE, (i + 1) * DH_SLICE)
        in_maps.append(
            {
                "qT": qT,
                "kT": kT,
                "vT": vT,
                "wqT": np.ascontiguousarray(wq_w[sl, :].T),
                "wkT": np.ascontiguousarray(wk_w[sl, :].T),
                "wvT": np.ascontiguousarray(wv_w[sl, :].T),
                "bq": np.ascontiguousarray(wq_b[sl].reshape(-1, 1)),
                "bk": np.ascontiguousarray(wk_b[sl].reshape(-1, 1)),
                "bv": np.ascontiguousarray(wv_b[sl].reshape(-1, 1)),
                "woT": np.ascontiguousarray(wo_w[:, sl].T),
            }
        )
    return in_maps


_NC_CACHE = {}


def _get_nc():
    if "nc" not in _NC_CACHE:
        _NC_CACHE["nc"] = build_mha_core()
    return _NC_CACHE["nc"]


def kernel(
    q,
    k,
    v,
    wq_w,
    wq_b,
    wk_w,
    wk_b,
    wv_w,
    wv_b,
    wo_w,
    wo_b,
    _trace: bool = False,
):
    from concourse.bass_utils import run_bass_kernel_spmd

    args = [np.asarray(x, dtype=np.float32) for x in (q, k, v)]
    wargs = [
        np.asarray(x, dtype=np.float32)
        for x in (wq_w, wq_b, wk_w, wk_b, wv_w, wv_b, wo_w)
    ]
    nc = _get_nc()
    in_maps = make_in_maps(*args, *wargs)
    res = run_bass_kernel_spmd(
        nc, in_maps, core_ids=list(range(N_CORES)), trace=_trace
    )
    out = np.zeros((SEQ, D_MODEL), np.float32)
    attn = np.empty((1, NUM_HEADS, SEQ, SEQ), np.float32)
    for i in range(N_CORES):
        out += res.results[i]["partial"]
        attn[0, i * HPC : (i + 1) * HPC] = res.results[i]["attn_out"]
    out += np.asarray(wo_b, np.float32)[None, :]
    out = out[None]  # [1, S, D]
    if _trace:
        kernel.last_results = res
    return out, attn


# revision 21
# speedup vs baseline: 1.1214x; 1.1214x over previous
"""Multi-head attention (16 heads, S=4096, D=1024) on 8 TRN2 NeuronCores.

Megatron-style tensor parallelism over heads: core i owns heads (2i, 2i+1).
Each core computes its head slice of the q/k/v projections, full attention
for its 2 heads (writing the softmax probabilities, which are part of the
module output), and a rank-128 partial of the output projection. The host
sums the 8 partials (the "all-reduce") and concatenates the attention
probability slices.

Device algorithm per core (matmuls in bf16, softmax/normalization fp32):
  qhT[dh,s]  = (wq_slice @ q^T)            (dh = 128 = 2 heads x 64)
  khT, vhT   likewise; vh_aug[sk,65]       = [vh | 1] per head (PE transpose)
  pass A (per head h, sq-block j of 512):
      for sk-tile t: LT = khT_t^T-dot-qhT_j -> exp(LT/tau)            (ACT)
                     ctxT[65,512] += [vh|1]^T @ exp  (PSUM accumulate)
      row 64 of ctxT = rowsum; transposed into per-sq-partition columns
      of 1/rowsum (DVE recip + PE transpose)
  pass B (per head h, sq-tile jj of 128):
      logits = qhT_jj^T-dot-khT -> exp(logits/tau)                    (ACT)
      -> attn = exp * (1/rowsum)[sq]  (DVE per-partition scale) -> DMA
  partial[s,:] = sum_h (ctxT_h_s^T @ woT_h) * (1/rowsum_h)[s]
"""

import sys

sys.path.insert(0, "/opt/trn_rl_repo")

import ml_dtypes
import numpy as np

import concourse.mybir as mybir
from concourse import bacc
from concourse.masks import make_identity
from concourse.tile import TileContext

F32 = mybir.dt.float32
BF16 = mybir.dt.bfloat16
AF = mybir.ActivationFunctionType

D_MODEL = 1024
NUM_HEADS = 16
DEPTH = 64
TAU = 8.0
SEQ = 4096
N_CORES = 8
HPC = NUM_HEADS // N_CORES  # heads per core = 2
DH_SLICE = HPC * DEPTH  # 128 output dims per core


def build_mha_core(seq: int = SEQ, d_model: int = D_MODEL):
    """Build the per-core Bass module (same SPMD program on all 8 cores)."""
    S, D = seq, d_model
    KB = D // 128          # contraction blocks for projections
    NJ = S // 512          # sq blocks (pass A)
    NT = S // 128          # sk tiles (pass A) == sq tiles (pass B)
    NM = S // 512          # sk blocks (pass B)
    SCALE = 1.0 / TAU
    assert NT % 2 == 0

    nc = bacc.Bacc("TRN2", target_bir_lowering=False)

    qT = nc.dram_tensor("qT", [D, S], BF16, kind="ExternalInput")
    kT = nc.dram_tensor("kT", [D, S], BF16, kind="ExternalInput")
    vT = nc.dram_tensor("vT", [D, S], BF16, kind="ExternalInput")
    wqT = nc.dram_tensor("wqT", [D, 128], BF16, kind="ExternalInput")
    wkT = nc.dram_tensor("wkT", [D, 128], BF16, kind="ExternalInput")
    wvT = nc.dram_tensor("wvT", [D, 128], BF16, kind="ExternalInput")
    bq = nc.dram_tensor("bq", [128, 1], F32, kind="ExternalInput")
    bk = nc.dram_tensor("bk", [128, 1], F32, kind="ExternalInput")
    bv = nc.dram_tensor("bv", [128, 1], F32, kind="ExternalInput")
    woT = nc.dram_tensor("woT", [128, D], BF16, kind="ExternalInput")
    attn_out = nc.dram_tensor("attn_out", [HPC, S, S], F32, kind="ExternalOutput")
    partial = nc.dram_tensor("partial", [S, D], F32, kind="ExternalOutput")

    with TileContext(nc) as tc:
        with (
            tc.tile_pool(name="consts", bufs=1) as consts,
            tc.tile_pool(name="persist", bufs=1) as persist,
            tc.tile_pool(name="xin", bufs=4) as xin,
            tc.tile_pool(name="expp", bufs=4) as expp,
            tc.tile_pool(name="attnb", bufs=2) as attnb,
            tc.tile_pool(name="outb", bufs=2) as outb,
            tc.tile_pool(name="lnp", bufs=2) as lnp,
            tc.tile_pool(name="ps_big", bufs=2, space="PSUM") as ps_big,
            tc.tile_pool(name="ps_ctx", bufs=2, space="PSUM") as ps_ctx,
            tc.tile_pool(name="ps_mm", bufs=2, space="PSUM") as ps_mm,
        ):
            # ---- constants -------------------------------------------------
            ident = consts.tile([128, 128], F32)
            make_identity(nc, ident)
            identb = consts.tile([128, 128], BF16)
            make_identity(nc, identb)

            w_sb = {}
            b_sb = {}
            for name, wdram, bdram in (
                ("q", wqT, bq),
                ("k", wkT, bk),
                ("v", wvT, bv),
            ):
                w = consts.tile([128, D], BF16, tag=f"w{name}")
                for kb in range(KB):
                    nc.sync.dma_start(
                        out=w[:, kb * 128 : (kb + 1) * 128],
                        in_=wdram[kb * 128 : (kb + 1) * 128, :],
                    )
                w_sb[name] = w
                b = consts.tile([128, 1], F32, tag=f"b{name}")
                nc.sync.dma_start(out=b, in_=bdram[:, :])
                b_sb[name] = b
            # woT rows for head h at partition base 0: [64, HPC, D]
            woT_sb = consts.tile([64, HPC, D], BF16)
            for h in range(HPC):
                nc.sync.dma_start(
                    out=woT_sb[:, h, :], in_=woT[h * DEPTH : (h + 1) * DEPTH, :]
                )

            # ---- persistent tensors ---------------------------------------
            qhT_sb = persist.tile([128, S], BF16)
            khT_sb = persist.tile([128, S], BF16)
            vhT_sb = persist.tile([128, S], BF16)
            vh_aug = persist.tile([128, HPC, NT, DEPTH + 1], BF16)
            # per-head unnormalized ctx^T
            ctxT_h = [
                persist.tile([64, S], BF16, tag=f"ctxT{h}", name=f"ctxT{h}")
                for h in range(HPC)
            ]
            # 1/rowsum as per-sq-partition columns, fp32
            recip_cols = persist.tile([128, HPC, NT], F32)

            # ---- phase 1: projections -> qhT/khT/vhT [128, S] -------------
            for name, xdram, dest in (
                ("k", kT, khT_sb),
                ("q", qT, qhT_sb),
                ("v", vT, vhT_sb),
            ):
                w = w_sb[name]
                for n in range(S // 512):
                    ps = ps_mm.tile([128, 512], F32, tag="mm512")
                    for kb in range(KB):
                        xt = xin.tile([128, 512], BF16, tag="xin")
                        nc.sync.dma_start(
                            out=xt,
                            in_=xdram[
                                kb * 128 : (kb + 1) * 128, n * 512 : (n + 1) * 512
                            ],
                        )
                        nc.tensor.matmul(
                            ps,
                            lhsT=w[:, kb * 128 : (kb + 1) * 128],
                            rhs=xt,
                            start=(kb == 0),
                            stop=(kb == KB - 1),
                        )
                    # copy psum -> sbuf with per-partition bias add
                    nc.vector.tensor_scalar_add(
                        dest[:, n * 512 : (n + 1) * 512], ps, b_sb[name]
                    )

            # ---- phase 1.5: vh_aug = [vh | 1] per head --------------------
            for h in range(HPC):
                hs = slice(h * DEPTH, (h + 1) * DEPTH)
                for t in range(NT):
                    nc.gpsimd.memset(vh_aug[:, h, t, DEPTH : DEPTH + 1], 1.0)
                    pst = ps_mm.tile([128, DEPTH], BF16, tag="mm512")
                    nc.tensor.transpose(
                        pst,
                        vhT_sb[hs, t * 128 : (t + 1) * 128],
                        identb[hs, hs],
                    )
                    nc.vector.tensor_copy(vh_aug[:, h, t, 0:DEPTH], pst)

            # ---- phase 2: attention ---------------------------------------
            for j in range(NJ):
                js = slice(j * 512, (j + 1) * 512)
                # pass A for both heads: ctxT accumulation + rowsums
                for h in range(HPC):
                    hs = slice(h * DEPTH, (h + 1) * DEPTH)
                    psc = ps_ctx.tile([DEPTH + 1, 512], F32, tag="ctx")
                    for tp in range(NT // 2):
                        psl = ps_big.tile([128, 1024], F32, tag="big")
                        for ti in range(2):
                            t = 2 * tp + ti
                            nc.tensor.matmul(
                                psl[:, ti * 512 : (ti + 1) * 512],
                                lhsT=khT_sb[hs, t * 128 : (t + 1) * 128],
                                rhs=qhT_sb[hs, js],
                                start=True,
                                stop=True,
                            )
                        ex = expp.tile([128, 1024], BF16, tag="exp")
                        nc.scalar.activation(ex, psl, AF.Exp, scale=SCALE)
                        for ti in range(2):
                            t = 2 * tp + ti
                            nc.tensor.matmul(
                                psc,
                                lhsT=vh_aug[:, h, t, :],
                                rhs=ex[:, ti * 512 : (ti + 1) * 512],
                                start=(t == 0),
                                stop=(t == NT - 1),
                            )
                    nc.vector.tensor_copy(ctxT_h[h][:, js], psc[0:DEPTH, :])
                    # 1/rowsum (fp32), transposed into per-partition columns
                    rn = lnp.tile([65, 512], F32, tag="rn")
                    nc.vector.reciprocal(
                        rn[64:65, :], psc[DEPTH : DEPTH + 1, :]
                    )
                    for c in range(4):
                        pst = ps_mm.tile([128, 1], F32, tag="mm512")
                        nc.tensor.transpose(
                            pst,
                            rn[64:65, c * 128 : (c + 1) * 128],
                            ident[64:65, 64:65],
                        )
                        nc.vector.tensor_copy(
                            recip_cols[:, h, j * 4 + c : j * 4 + c + 1], pst
                        )
                # pass B: probabilities -> DRAM
                for h in range(HPC):
                    hs = slice(h * DEPTH, (h + 1) * DEPTH)
                    for c in range(4):
                        jj = j * 4 + c
                        ab = attnb.tile([128, S], F32, tag="attn")
                        for base in range(0, NM, 2):
                            wdt = min(2, NM - base)
                            pslg = ps_big.tile([128, 512 * wdt], F32, tag="big")
                            for mi in range(wdt):
                                m = base + mi
                                nc.tensor.matmul(
                                    pslg[:, mi * 512 : (mi + 1) * 512],
                                    lhsT=qhT_sb[hs, jj * 128 : (jj + 1) * 128],
                                    rhs=khT_sb[hs, m * 512 : (m + 1) * 512],
                                    start=True,
                                    stop=True,
                                )
                            nc.scalar.activation(
                                ab[:, base * 512 : (base + wdt) * 512],
                                pslg,
                                AF.Exp,
                                scale=SCALE,
                            )
                        # normalize in one per-partition scale over the row
                        nc.vector.tensor_scalar_mul(
                            ab, ab, recip_cols[:, h, jj : jj + 1]
                        )
                        nc.sync.dma_start(
                            out=attn_out[h, jj * 128 : (jj + 1) * 128, :], in_=ab
                        )

            # ---- phase 3: output projection -------------------------------
            for st in range(NT):
                ob = outb.tile([128, D], F32, tag="ob")
                for dhf in range(D // 512):
                    dsl = slice(dhf * 512, (dhf + 1) * 512)
                    pso = []
                    for h in range(HPC):
                        p = ps_mm.tile([128, 512], F32, tag="mm512", name=f"pso{h}")
                        nc.tensor.matmul(
                            p,
                            lhsT=ctxT_h[h][:, st * 128 : (st + 1) * 128],
                            rhs=woT_sb[:, h, dsl],
                            start=True,
                            stop=True,
                        )
                        pso.append(p)
                    nc.vector.tensor_scalar_mul(
                        ob[:, dsl], pso[0], recip_cols[:, 0, st : st + 1]
                    )
                    nc.vector.scalar_tensor_tensor(
                        out=ob[:, dsl],
                        in0=pso[1],
                        scalar=recip_cols[:, 1, st : st + 1],
                        in1=ob[:, dsl],
                        op0=mybir.AluOpType.mult,
                        op1=mybir.AluOpType.add,
                    )
                nc.sync.dma_start(out=partial[st * 128 : (st + 1) * 128, :], in_=ob)

    nc.compile()
    return nc


def make_in_maps(q, k, v, wq_w, wq_b, wk_w, wk_b, wv_w, wv_b, wo_w):
    bf = ml_dtypes.bfloat16
    qT = np.ascontiguousarray(q.T).astype(bf)
    kT = np.ascontiguousarray(k.T).astype(bf)
    vT = np.ascontiguousarray(v.T).astype(bf)
    in_maps = []
    for i in range(N_CORES):
        sl = slice(i * DH_SLICE, (i + 1) * DH_SLICE)
        in_maps.append(
            {
                "qT": qT,
                "kT": kT,
                "vT": vT,
                "wqT": np.ascontiguousarray(wq_w[sl, :].T).astype(bf),
                "wkT": np.ascontiguousarray(wk_w[sl, :].T).astype(bf),
                "wvT": np.ascontiguousarray(wv_w[sl, :].T).astype(bf),
                "bq": np.ascontiguousarray(wq_b[sl].reshape(-1, 1), dtype=np.float32),
                "bk": np.ascontiguousarray(wk_b[sl].reshape(-1, 1), dtype=np.float32),
                "bv": np.ascontiguousarray(wv_b[sl].reshape(-1, 1), dtype=np.float32),
                "woT": np.ascontiguousarray(wo_w[:, sl].T).astype(bf),
            }
        )
    return in_maps


_NC_CACHE = {}


def _get_nc():
    if "nc" not in _NC_CACHE:
        _NC_CACHE["nc"] = build_mha_core()
    return _NC_CACHE["nc"]


def kernel(
    q,
    k,
    v,
    wq_w,
    wq_b,
    wk_w,
    wk_b,
    wv_w,
    wv_b,
    wo_w,
    wo_b,
    _trace: bool = False,
):
    from concourse.bass_utils import run_bass_kernel_spmd

    args = [np.asarray(x, dtype=np.float32) for x in (q, k, v)]
    wargs = [
        np.asarray(x, dtype=np.float32)
        for x in (wq_w, wq_b, wk_w, wk_b, wv_w, wv_b, wo_w)
    ]
    nc = _get_nc()
    in_maps = make_in_maps(*args, *wargs)
    res = run_bass_kernel_spmd(
        nc, in_maps, core_ids=list(range(N_CORES)), trace=_trace
    )
    out = np.zeros((SEQ, D_MODEL), np.float32)
    attn = np.empty((1, NUM_HEADS, SEQ, SEQ), np.float32)
    for i in range(N_CORES):
        out += res.results[i]["partial"]
        attn[0, i * HPC : (i + 1) * HPC] = res.results[i]["attn_out"]
    out += np.asarray(wo_b, np.float32)[None, :]
    out = out[None]  # [1, S, D]
    if _trace:
        kernel.last_results = res
    return out, attn


# revision 22
# speedup vs baseline: 1.2427x; 1.1081x over previous
"""Multi-head attention (16 heads, S=4096, D=1024) on 8 TRN2 NeuronCores.

Megatron-style tensor parallelism over heads: core i owns heads (2i, 2i+1).
Each core computes its head slice of the q/k/v projections, full attention
for its 2 heads (writing the softmax probabilities, which are part of the
module output), and a rank-128 partial of the output projection. The host
sums the 8 partials (the "all-reduce") and concatenates the attention
probability slices.

Device algorithm per core (matmuls in bf16, softmax/normalization fp32):
  qhT[dh,s]  = (wq_slice @ q^T)            (dh = 128 = 2 heads x 64)
  khT, vhT   likewise; vh_aug[sk,65]       = [vh | 1] per head (PE transpose)
  pass A (per head h, sq-block j of 512):
      for sk-tile t: LT = khT_t^T-dot-qhT_j -> exp(LT/tau)            (ACT)
                     ctxT[65,512] += [vh|1]^T @ exp  (PSUM accumulate)
      row 64 of ctxT = rowsum; transposed into per-sq-partition columns
      of 1/rowsum (DVE recip + PE transpose)
  pass B (per head h, sq-tile jj of 128):
      logits = qhT_jj^T-dot-khT -> exp(logits/tau)                    (ACT)
      -> attn = exp * (1/rowsum)[sq]  (DVE per-partition scale) -> DMA
  partial[s,:] = sum_h (ctxT_h_s^T @ woT_h) * (1/rowsum_h)[s]
"""

import sys

sys.path.insert(0, "/opt/trn_rl_repo")

import ml_dtypes
import numpy as np

import concourse.mybir as mybir
from concourse import bacc
from concourse.masks import make_identity
from concourse.tile import TileContext

F32 = mybir.dt.float32
BF16 = mybir.dt.bfloat16
AF = mybir.ActivationFunctionType

D_MODEL = 1024
NUM_HEADS = 16
DEPTH = 64
TAU = 8.0
SEQ = 4096
N_CORES = 8
HPC = NUM_HEADS // N_CORES  # heads per core = 2
DH_SLICE = HPC * DEPTH  # 128 output dims per core


def build_mha_core(seq: int = SEQ, d_model: int = D_MODEL):
    """Build the per-core Bass module (same SPMD program on all 8 cores)."""
    S, D = seq, d_model
    KB = D // 128          # contraction blocks for projections
    NJ = S // 512          # sq blocks (pass A)
    NT = S // 128          # sk tiles (pass A) == sq tiles (pass B)
    NM = S // 512          # sk blocks (pass B)
    SCALE = 1.0 / TAU
    assert NT % 2 == 0

    nc = bacc.Bacc("TRN2", target_bir_lowering=False)

    qT = nc.dram_tensor("qT", [D, S], BF16, kind="ExternalInput")
    kT = nc.dram_tensor("kT", [D, S], BF16, kind="ExternalInput")
    vT = nc.dram_tensor("vT", [D, S], BF16, kind="ExternalInput")
    wqT = nc.dram_tensor("wqT", [D, 128], BF16, kind="ExternalInput")
    wkT = nc.dram_tensor("wkT", [D, 128], BF16, kind="ExternalInput")
    wvT = nc.dram_tensor("wvT", [D, 128], BF16, kind="ExternalInput")
    bq = nc.dram_tensor("bq", [128, 1], F32, kind="ExternalInput")
    bk = nc.dram_tensor("bk", [128, 1], F32, kind="ExternalInput")
    bv = nc.dram_tensor("bv", [128, 1], F32, kind="ExternalInput")
    woT = nc.dram_tensor("woT", [128, D], BF16, kind="ExternalInput")
    attn_out = nc.dram_tensor("attn_out", [HPC, S, S], F32, kind="ExternalOutput")
    partial = nc.dram_tensor("partial", [S, D], F32, kind="ExternalOutput")

    with TileContext(nc) as tc:
        with (
            tc.tile_pool(name="consts", bufs=1) as consts,
            tc.tile_pool(name="persist", bufs=1) as persist,
            tc.tile_pool(name="xin", bufs=4) as xin,
            tc.tile_pool(name="expp", bufs=4) as expp,
            tc.tile_pool(name="attnb", bufs=2) as attnb,
            tc.tile_pool(name="outb", bufs=2) as outb,
            tc.tile_pool(name="lnp", bufs=2) as lnp,
            tc.tile_pool(name="ps_a", bufs=2, space="PSUM") as ps_a,
            tc.tile_pool(name="ps_b", bufs=2, space="PSUM") as ps_b,
            tc.tile_pool(name="ps_ctx", bufs=2, space="PSUM") as ps_ctx,
        ):
            # ---- constants -------------------------------------------------
            ident = consts.tile([128, 128], F32)
            make_identity(nc, ident)
            identb = consts.tile([128, 128], BF16)
            make_identity(nc, identb)

            w_sb = {}
            b_sb = {}
            for name, wdram, bdram in (
                ("q", wqT, bq),
                ("k", wkT, bk),
                ("v", wvT, bv),
            ):
                w = consts.tile([128, D], BF16, tag=f"w{name}")
                for kb in range(KB):
                    nc.sync.dma_start(
                        out=w[:, kb * 128 : (kb + 1) * 128],
                        in_=wdram[kb * 128 : (kb + 1) * 128, :],
                    )
                w_sb[name] = w
                b = consts.tile([128, 1], F32, tag=f"b{name}")
                nc.sync.dma_start(out=b, in_=bdram[:, :])
                b_sb[name] = b
            # woT rows for head h at partition base 0: [64, HPC, D]
            woT_sb = consts.tile([64, HPC, D], BF16)
            for h in range(HPC):
                nc.sync.dma_start(
                    out=woT_sb[:, h, :], in_=woT[h * DEPTH : (h + 1) * DEPTH, :]
                )

            # ---- persistent tensors ---------------------------------------
            qhT_sb = persist.tile([128, S], BF16)
            khT_sb = persist.tile([128, S], BF16)
            vhT_sb = persist.tile([128, S], BF16)
            vh_aug = persist.tile([128, HPC, NT, DEPTH + 1], BF16)
            # per-head unnormalized ctx^T
            ctxT_h = [
                persist.tile([64, S], BF16, tag=f"ctxT{h}", name=f"ctxT{h}")
                for h in range(HPC)
            ]
            # 1/rowsum as per-sq-partition columns, fp32
            recip_cols = persist.tile([128, HPC, NT], F32)

            # ---- phase 1: projections -> qhT/khT/vhT [128, S] -------------
            for name, xdram, dest in (
                ("k", kT, khT_sb),
                ("q", qT, qhT_sb),
                ("v", vT, vhT_sb),
            ):
                w = w_sb[name]
                for n in range(S // 512):
                    ps = ps_ctx.tile([128, 512], F32, tag="ctx", name="ps")
                    for kb in range(KB):
                        xt = xin.tile([128, 512], BF16, tag="xin")
                        nc.sync.dma_start(
                            out=xt,
                            in_=xdram[
                                kb * 128 : (kb + 1) * 128, n * 512 : (n + 1) * 512
                            ],
                        )
                        nc.tensor.matmul(
                            ps,
                            lhsT=w[:, kb * 128 : (kb + 1) * 128],
                            rhs=xt,
                            start=(kb == 0),
                            stop=(kb == KB - 1),
                        )
                    # copy psum -> sbuf with per-partition bias add
                    nc.vector.tensor_scalar_add(
                        dest[:, n * 512 : (n + 1) * 512], ps, b_sb[name]
                    )

            # ---- phase 1.5: vh_aug = [vh | 1] per head --------------------
            for h in range(HPC):
                hs = slice(h * DEPTH, (h + 1) * DEPTH)
                for t in range(NT):
                    nc.gpsimd.memset(vh_aug[:, h, t, DEPTH : DEPTH + 1], 1.0)
                    pst = ps_ctx.tile([128, DEPTH], BF16, tag="ctx", name="pst")
                    nc.tensor.transpose(
                        pst,
                        vhT_sb[hs, t * 128 : (t + 1) * 128],
                        identb[hs, hs],
                    )
                    nc.vector.tensor_copy(vh_aug[:, h, t, 0:DEPTH], pst)

            # ---- phase 2: attention ---------------------------------------
            # Software-pipelined: pass A (both heads interleaved) for block j
            # runs with pass B work for block j-1 woven in, so the PE stream
            # has long dependency-free runs and ACT (exp) stays saturated.
            ab_tiles = {}

            def passB_units(jprev):
                units = []
                for h in range(HPC):
                    for c in range(4):
                        jj = jprev * 4 + c
                        for base in range(0, NM, 2):
                            units.append((h, jj, base, min(2, NM - base)))
                return units

            def emit_passB_unit(unit):
                h, jj, base, wdt = unit
                hs = slice(h * DEPTH, (h + 1) * DEPTH)
                if base == 0:
                    ab_tiles[(h, jj)] = attnb.tile(
                        [128, S], F32, tag="attn", name=f"ab{h}_{jj}"
                    )
                ab = ab_tiles[(h, jj)]
                pslg = ps_b.tile([128, 512 * wdt], F32, tag="big", name="pslg")
                for mi in range(wdt):
                    m = base + mi
                    nc.tensor.matmul(
                        pslg[:, mi * 512 : (mi + 1) * 512],
                        lhsT=qhT_sb[hs, jj * 128 : (jj + 1) * 128],
                        rhs=khT_sb[hs, m * 512 : (m + 1) * 512],
                        start=True,
                        stop=True,
                    )
                nc.scalar.activation(
                    ab[:, base * 512 : (base + wdt) * 512],
                    pslg,
                    AF.Exp,
                    scale=SCALE,
                )
                if base + wdt == NM:
                    nc.vector.tensor_scalar_mul(
                        ab, ab, recip_cols[:, h, jj : jj + 1]
                    )
                    nc.sync.dma_start(
                        out=attn_out[h, jj * 128 : (jj + 1) * 128, :], in_=ab
                    )
                    del ab_tiles[(h, jj)]

            def emit_mmA(h, j, t, psl):
                hs = slice(h * DEPTH, (h + 1) * DEPTH)
                nc.tensor.matmul(
                    psl,
                    lhsT=khT_sb[hs, t * 128 : (t + 1) * 128],
                    rhs=qhT_sb[hs, j * 512 : (j + 1) * 512],
                    start=True,
                    stop=True,
                )

            def emit_exp1(psl):
                ex = expp.tile([128, 512], BF16, tag="exp")
                nc.scalar.activation(ex, psl, AF.Exp, scale=SCALE)
                return ex

            def emit_mmB(h, t, ex, psc):
                nc.tensor.matmul(
                    psc,
                    lhsT=vh_aug[:, h, t, :],
                    rhs=ex,
                    start=(t == 0),
                    stop=(t == NT - 1),
                )

            def emit_ctx_epilogue(h, j, psc):
                js = slice(j * 512, (j + 1) * 512)
                nc.vector.tensor_copy(ctxT_h[h][:, js], psc[0:DEPTH, :])
                rn = lnp.tile([65, 512], F32, tag="rn")
                nc.vector.reciprocal(rn[64:65, :], psc[DEPTH : DEPTH + 1, :])
                for c in range(4):
                    pst = ps_ctx.tile([128, 1], F32, tag="ctx", name="pst")
                    nc.tensor.transpose(
                        pst,
                        rn[64:65, c * 128 : (c + 1) * 128],
                        ident[64:65, 64:65],
                    )
                    nc.vector.tensor_copy(
                        recip_cols[:, h, j * 4 + c : j * 4 + c + 1], pst
                    )

            for j in range(NJ):
                units = passB_units(j - 1) if j > 0 else []
                U = len(units)
                psc = [
                    ps_ctx.tile([DEPTH + 1, 512], F32, tag="ctx", name=f"psc{h}")
                    for h in range(HPC)
                ]
                prev = {}
                for t in range(NT):
                    cur = {}
                    for h in range(HPC):
                        psl = ps_a.tile([128, 512], F32, tag="lt", name="psl")
                        emit_mmA(h, j, t, psl)
                        cur[h] = psl
                    # pass-B filler: no fresh-exp dependency, absorbs stalls
                    for k in range(t * U // NT, (t + 1) * U // NT):
                        emit_passB_unit(units[k])
                    for h in range(HPC):
                        ex = emit_exp1(cur[h])
                        cur[h] = ex
                    if t > 0:
                        for h in range(HPC):
                            emit_mmB(h, t - 1, prev[h], psc[h])
                    prev = cur
                for h in range(HPC):
                    emit_mmB(h, NT - 1, prev[h], psc[h])
                for h in range(HPC):
                    emit_ctx_epilogue(h, j, psc[h])

            # ---- phase 3: output projection, interleaved with the last
            # j-block's pass B -----------------------------------------------
            units = passB_units(NJ - 1)
            U = len(units)
            for st in range(NT):
                for k in range(st * U // NT, (st + 1) * U // NT):
                    emit_passB_unit(units[k])
                ob = outb.tile([128, D], F32, tag="ob")
                for dhf in range(D // 512):
                    dsl = slice(dhf * 512, (dhf + 1) * 512)
                    pso = []
                    for h in range(HPC):
                        p = ps_ctx.tile([128, 512], F32, tag="ctx", name=f"pso{h}")
                        nc.tensor.matmul(
                            p,
                            lhsT=ctxT_h[h][:, st * 128 : (st + 1) * 128],
                            rhs=woT_sb[:, h, dsl],
                            start=True,
                            stop=True,
                        )
                        pso.append(p)
                    nc.vector.tensor_scalar_mul(
                        ob[:, dsl], pso[0], recip_cols[:, 0, st : st + 1]
                    )
                    nc.vector.scalar_tensor_tensor(
                        out=ob[:, dsl],
                        in0=pso[1],
                        scalar=recip_cols[:, 1, st : st + 1],
                        in1=ob[:, dsl],
                        op0=mybir.AluOpType.mult,
                        op1=mybir.AluOpType.add,
                    )
                nc.sync.dma_start(out=partial[st * 128 : (st + 1) * 128, :], in_=ob)

    nc.compile()
    return nc


def make_in_maps(q, k, v, wq_w, wq_b, wk_w, wk_b, wv_w, wv_b, wo_w):
    bf = ml_dtypes.bfloat16
    qT = np.ascontiguousarray(q.T).astype(bf)
    kT = np.ascontiguousarray(k.T).astype(bf)
    vT = np.ascontiguousarray(v.T).astype(bf)
    in_maps = []
    for i in range(N_CORES):
        sl = slice(i * DH_SLICE, (i + 1) * DH_SLICE)
        in_maps.append(
            {
                "qT": qT,
                "kT": kT,
                "vT": vT,
                "wqT": np.ascontiguousarray(wq_w[sl, :].T).astype(bf),
                "wkT": np.ascontiguousarray(wk_w[sl, :].T).astype(bf),
                "wvT": np.ascontiguousarray(wv_w[sl, :].T).astype(bf),
                "bq": np.ascontiguousarray(wq_b[sl].reshape(-1, 1), dtype=np.float32),
                "bk": np.ascontiguousarray(wk_b[sl].reshape(-1, 1), dtype=np.float32),
                "bv": np.ascontiguousarray(wv_b[sl].reshape(-1, 1), dtype=np.float32),
                "woT": np.ascontiguousarray(wo_w[:, sl].T).astype(bf),
            }
        )
    return in_maps


_NC_CACHE = {}


def _get_nc():
    if "nc" not in _NC_CACHE:
        _NC_CACHE["nc"] = build_mha_core()
    return _NC_CACHE["nc"]


def kernel(
    q,
    k,
    v,
    wq_w,
    wq_b,
    wk_w,
    wk_b,
    wv_w,
    wv_b,
    wo_w,
    wo_b,
    _trace: bool = False,
):
    from concourse.bass_utils import run_bass_kernel_spmd

    args = [np.asarray(x, dtype=np.float32) for x in (q, k, v)]
    wargs = [
        np.asarray(x, dtype=np.float32)
        for x in (wq_w, wq_b, wk_w, wk_b, wv_w, wv_b, wo_w)
    ]
    nc = _get_nc()
    in_maps = make_in_maps(*args, *wargs)
    res = run_bass_kernel_spmd(
        nc, in_maps, core_ids=list(range(N_CORES)), trace=_trace
    )
    out = np.zeros((SEQ, D_MODEL), np.float32)
    attn = np.empty((1, NUM_HEADS, SEQ, SEQ), np.float32)
    for i in range(N_CORES):
        out += res.results[i]["partial"]
        attn[0, i * HPC : (i + 1) * HPC] = res.results[i]["attn_out"]
    out += np.asarray(wo_b, np.float32)[None, :]
    out = out[None]  # [1, S, D]
    if _trace:
        kernel.last_results = res
    return out, attn


# revision 24
# speedup vs baseline: 1.5124x; 1.2171x over previous
"""Multi-head attention (16 heads, S=4096, D=1024) on 8 TRN2 NeuronCores.

Megatron-style tensor parallelism over heads: core i owns heads (2i, 2i+1).
Each core computes its head slice of the q/k/v projections, full attention
for its 2 heads (writing the softmax probabilities, which are part of the
module output), and a rank-128 partial of the output projection. The host
sums the 8 partials (the "all-reduce") and concatenates the attention
probability slices.

Device algorithm per core (matmuls bf16, softmax/normalization fp32):
  qhT[dh,s] = wq_slice @ q^T  (dh = 128 = 2 heads x 64); same khT, vhT.
  vh[sk,64] per head via PE transpose.
  pass A (per sq-block j of 512, heads row/col-packed on the PE):
    for sk-tile t: LT(h) = khT_t^T-dot-qhT_j  (2 heads as concurrent 64-row
    PE tiles into one [128,1024] PSUM pair) -> exp(LT/tau) (one ACT op)
    -> ctx^T[128,512] += vh^T @ exp  (2 heads as concurrent 64-col PE tiles)
  pass B (per sq-tile jj of 128, independent of pass A):
    logits = qhT_jj^T-dot-khT -> exp(logits/tau) with ACT accum_out giving
    the rowsum -> attn = exp * (1/rowsum)[sq] (DVE) -> DMA
  partial[s,:] = sum_h (ctx_h^T @ woT_h) * (1/rowsum_h)[s]  (PE row-packed)
"""

import sys

sys.path.insert(0, "/opt/trn_rl_repo")

import ml_dtypes
import numpy as np

import concourse.mybir as mybir
from concourse import bacc
from concourse.masks import make_identity
from concourse.tile import TileContext

F32 = mybir.dt.float32
BF16 = mybir.dt.bfloat16
AF = mybir.ActivationFunctionType

D_MODEL = 1024
NUM_HEADS = 16
DEPTH = 64
TAU = 8.0
SEQ = 4096
N_CORES = 8
HPC = NUM_HEADS // N_CORES  # heads per core = 2
DH_SLICE = HPC * DEPTH  # 128 output dims per core


def build_mha_core(seq: int = SEQ, d_model: int = D_MODEL):
    """Build the per-core Bass module (same SPMD program on all 8 cores)."""
    S, D = seq, d_model
    KB = D // 128          # contraction blocks for projections
    NJ = S // 512          # sq blocks (pass A)
    NT = S // 128          # sk tiles (pass A) == sq tiles (pass B)
    NM = S // 512          # sk blocks (pass B)
    SCALE = 1.0 / TAU

    nc = bacc.Bacc("TRN2", target_bir_lowering=False)

    qT = nc.dram_tensor("qT", [D, S], BF16, kind="ExternalInput")
    kT = nc.dram_tensor("kT", [D, S], BF16, kind="ExternalInput")
    vT = nc.dram_tensor("vT", [D, S], BF16, kind="ExternalInput")
    wqT = nc.dram_tensor("wqT", [D, 128], BF16, kind="ExternalInput")
    wkT = nc.dram_tensor("wkT", [D, 128], BF16, kind="ExternalInput")
    wvT = nc.dram_tensor("wvT", [D, 128], BF16, kind="ExternalInput")
    bq = nc.dram_tensor("bq", [128, 1], F32, kind="ExternalInput")
    bk = nc.dram_tensor("bk", [128, 1], F32, kind="ExternalInput")
    bv = nc.dram_tensor("bv", [128, 1], F32, kind="ExternalInput")
    woT = nc.dram_tensor("woT", [128, D], BF16, kind="ExternalInput")
    attn_out = nc.dram_tensor("attn_out", [HPC, S, S], F32, kind="ExternalOutput")
    partial = nc.dram_tensor("partial", [S, D], F32, kind="ExternalOutput")

    with TileContext(nc) as tc:
        with (
            tc.tile_pool(name="consts", bufs=1) as consts,
            tc.tile_pool(name="persist", bufs=1) as persist,
            tc.tile_pool(name="xin", bufs=4) as xin,
            tc.tile_pool(name="expp", bufs=4) as expp,
            tc.tile_pool(name="attnb", bufs=4) as attnb,
            tc.tile_pool(name="outb", bufs=2) as outb,
            tc.tile_pool(name="rsp", bufs=4) as rsp,
            tc.tile_pool(name="ps_a", bufs=2, space="PSUM") as ps_a,
            tc.tile_pool(name="ps_b", bufs=2, space="PSUM") as ps_b,
            tc.tile_pool(name="ps_c", bufs=2, space="PSUM") as ps_c,
        ):
            # ---- constants -------------------------------------------------
            identb = consts.tile([128, 128], BF16)
            make_identity(nc, identb)

            w_sb = {}
            b_sb = {}
            for name, wdram, bdram in (
                ("q", wqT, bq),
                ("k", wkT, bk),
                ("v", wvT, bv),
            ):
                w = consts.tile([128, D], BF16, tag=f"w{name}")
                for kb in range(KB):
                    nc.sync.dma_start(
                        out=w[:, kb * 128 : (kb + 1) * 128],
                        in_=wdram[kb * 128 : (kb + 1) * 128, :],
                    )
                w_sb[name] = w
                b = consts.tile([128, 1], F32, tag=f"b{name}")
                nc.sync.dma_start(out=b, in_=bdram[:, :])
                b_sb[name] = b
            # woT rows for head h at partition base 0: [64, HPC, D]
            woT_sb = consts.tile([64, HPC, D], BF16)
            for h in range(HPC):
                nc.sync.dma_start(
                    out=woT_sb[:, h, :], in_=woT[h * DEPTH : (h + 1) * DEPTH, :]
                )

            # ---- persistent tensors ---------------------------------------
            qhT_sb = persist.tile([128, S], BF16)
            khT_sb = persist.tile([128, S], BF16)
            vhT_sb = persist.tile([128, S], BF16)
            vh_sb = persist.tile([128, HPC, NT, DEPTH], BF16)
            # per-head unnormalized ctx^T
            ctxT_h = [
                persist.tile([64, S], BF16, tag=f"ctxT{h}", name=f"ctxT{h}")
                for h in range(HPC)
            ]
            # 1/rowsum per head as per-sq-partition columns, fp32
            recip_cols = persist.tile([128, HPC, NT], F32)

            # ---- phase 1: projections -> qhT/khT/vhT [128, S] -------------
            for name, xdram, dest in (
                ("k", kT, khT_sb),
                ("q", qT, qhT_sb),
                ("v", vT, vhT_sb),
            ):
                w = w_sb[name]
                for n in range(S // 512):
                    ps = ps_c.tile([128, 512], F32, tag="psc", name="ps")
                    for kb in range(KB):
                        xt = xin.tile([128, 512], BF16, tag="xin")
                        nc.sync.dma_start(
                            out=xt,
                            in_=xdram[
                                kb * 128 : (kb + 1) * 128, n * 512 : (n + 1) * 512
                            ],
                        )
                        nc.tensor.matmul(
                            ps,
                            lhsT=w[:, kb * 128 : (kb + 1) * 128],
                            rhs=xt,
                            start=(kb == 0),
                            stop=(kb == KB - 1),
                        )
                    nc.vector.tensor_scalar_add(
                        dest[:, n * 512 : (n + 1) * 512], ps, b_sb[name]
                    )

            # ---- phase 1.5: vh per head (transposed) ----------------------
            for h in range(HPC):
                hs = slice(h * DEPTH, (h + 1) * DEPTH)
                for t in range(NT):
                    pst = ps_c.tile([128, DEPTH], BF16, tag="psc", name="pst")
                    nc.tensor.transpose(
                        pst,
                        vhT_sb[hs, t * 128 : (t + 1) * 128],
                        identb[hs, hs],
                    )
                    nc.vector.tensor_copy(vh_sb[:, h, t, :], pst)

            # ---- phase 2: attention ---------------------------------------
            ab_tiles = {}
            rs_tiles = {}

            def emit_passB_unit(h, jj, m):
                hs = slice(h * DEPTH, (h + 1) * DEPTH)
                if m == 0:
                    ab_tiles[h] = attnb.tile(
                        [128, S], F32, tag="attn", name=f"ab{h}"
                    )
                    rs_tiles[h] = rsp.tile([128, NM], F32, tag="rs", name=f"rs{h}")
                ab, rs = ab_tiles[h], rs_tiles[h]
                pslg = ps_b.tile([128, 512], F32, tag="lg", name="pslg")
                nc.tensor.matmul(
                    pslg,
                    lhsT=qhT_sb[hs, jj * 128 : (jj + 1) * 128],
                    rhs=khT_sb[hs, m * 512 : (m + 1) * 512],
                    start=True,
                    stop=True,
                )
                nc.scalar.activation(
                    ab[:, m * 512 : (m + 1) * 512],
                    pslg,
                    AF.Exp,
                    scale=SCALE,
                    accum_out=rs[:, m : m + 1],
                )
                if m == NM - 1:
                    rsum = rsp.tile([128, 1], F32, tag="rsum", name="rsum")
                    if NM > 1:
                        nc.vector.reduce_sum(
                            rsum, rs, axis=mybir.AxisListType.X
                        )
                    else:
                        rsum = rs
                    nc.vector.reciprocal(recip_cols[:, h, jj : jj + 1], rsum)
                    nc.vector.tensor_scalar_mul(
                        ab, ab, recip_cols[:, h, jj : jj + 1]
                    )
                    nc.sync.dma_start(
                        out=attn_out[h, jj * 128 : (jj + 1) * 128, :], in_=ab
                    )

            # flat list of pass-B work items, two per pass-A step
            passB = [
                (h, jj, m)
                for jj in range(NT)
                for m in range(NM)
                for h in range(HPC)
            ]
            UB = len(passB)
            TOT = NJ * NT
            done = 0

            for j in range(NJ):
                js = slice(j * 512, (j + 1) * 512)
                psc = [
                    ps_c.tile([64, 512], F32, tag="psc", name=f"psc{h}")
                    for h in range(HPC)
                ]
                prev = None
                for t in range(NT):
                    step = j * NT + t
                    # pass A: QK^T for both heads as concurrent 64-row tiles
                    psl = ps_a.tile([128, 1024], F32, tag="lt", name="psl")
                    for h in range(HPC):
                        hs = slice(h * DEPTH, (h + 1) * DEPTH)
                        nc.tensor.matmul(
                            psl[:, h * 512 : (h + 1) * 512],
                            lhsT=khT_sb[hs, t * 128 : (t + 1) * 128],
                            rhs=qhT_sb[hs, js],
                            start=True,
                            stop=True,
                        )
                    # pass-B filler (independent of fresh exps)
                    lim = (step + 1) * UB // TOT
                    while done < lim:
                        emit_passB_unit(*passB[done])
                        done += 1
                    ex = expp.tile([128, 1024], BF16, tag="exp")
                    nc.scalar.activation(ex, psl, AF.Exp, scale=SCALE)
                    if prev is not None:
                        pex, pt = prev
                        for h in range(HPC):
                            nc.tensor.matmul(
                                psc[h],
                                lhsT=vh_sb[:, h, pt, :],
                                rhs=pex[:, h * 512 : (h + 1) * 512],
                                start=(pt == 0),
                                stop=(pt == NT - 1),
                            )
                    prev = (ex, t)
                pex, pt = prev
                for h in range(HPC):
                    nc.tensor.matmul(
                        psc[h],
                        lhsT=vh_sb[:, h, pt, :],
                        rhs=pex[:, h * 512 : (h + 1) * 512],
                        start=(pt == 0),
                        stop=(pt == NT - 1),
                    )
                for h in range(HPC):
                    nc.vector.tensor_copy(ctxT_h[h][:, js], psc[h])

            # ---- phase 3: output projection (heads as 64-row PE tiles) ----
            for st in range(NT):
                ob = outb.tile([128, D], F32, tag="ob")
                for dhf in range(D // 512):
                    dsl = slice(dhf * 512, (dhf + 1) * 512)
                    pso = []
                    for h in range(HPC):
                        hs = slice(h * DEPTH, (h + 1) * DEPTH)
                        p = ps_a.tile([128, 512], F32, tag="lt", name=f"pso{h}")
                        nc.tensor.matmul(
                            p,
                            lhsT=ctxT_h[h][:, st * 128 : (st + 1) * 128],
                            rhs=woT_sb[:, h, dsl],
                            start=True,
                            stop=True,
                        )
                        pso.append(p)
                    nc.vector.tensor_scalar_mul(
                        ob[:, dsl], pso[0], recip_cols[:, 0, st : st + 1]
                    )
                    nc.vector.scalar_tensor_tensor(
                        out=ob[:, dsl],
                        in0=pso[1],
                        scalar=recip_cols[:, 1, st : st + 1],
                        in1=ob[:, dsl],
                        op0=mybir.AluOpType.mult,
                        op1=mybir.AluOpType.add,
                    )
                nc.sync.dma_start(out=partial[st * 128 : (st + 1) * 128, :], in_=ob)

    nc.compile()
    return nc


def make_in_maps(q, k, v, wq_w, wq_b, wk_w, wk_b, wv_w, wv_b, wo_w):
    bf = ml_dtypes.bfloat16
    qT = np.ascontiguousarray(q.T).astype(bf)
    kT = np.ascontiguousarray(k.T).astype(bf)
    vT = np.ascontiguousarray(v.T).astype(bf)
    in_maps = []
    for i in range(N_CORES):
        sl = slice(i * DH_SLICE, (i + 1) * DH_SLICE)
        in_maps.append(
            {
                "qT": qT,
                "kT": kT,
                "vT": vT,
                "wqT": np.ascontiguousarray(wq_w[sl, :].T).astype(bf),
                "wkT": np.ascontiguousarray(wk_w[sl, :].T).astype(bf),
                "wvT": np.ascontiguousarray(wv_w[sl, :].T).astype(bf),
                "bq": np.ascontiguousarray(wq_b[sl].reshape(-1, 1), dtype=np.float32),
                "bk": np.ascontiguousarray(wk_b[sl].reshape(-1, 1), dtype=np.float32),
                "bv": np.ascontiguousarray(wv_b[sl].reshape(-1, 1), dtype=np.float32),
                "woT": np.ascontiguousarray(wo_w[:, sl].T).astype(bf),
            }
        )
    return in_maps


_NC_CACHE = {}


def _get_nc():
    if "nc" not in _NC_CACHE:
        _NC_CACHE["nc"] = build_mha_core()
    return _NC_CACHE["nc"]


def kernel(
    q,
    k,
    v,
    wq_w,
    wq_b,
    wk_w,
    wk_b,
    wv_w,
    wv_b,
    wo_w,
    wo_b,
    _trace: bool = False,
):
    from concourse.bass_utils import run_bass_kernel_spmd

    args = [np.asarray(x, dtype=np.float32) for x in (q, k, v)]
    wargs = [
        np.asarray(x, dtype=np.float32)
        for x in (wq_w, wq_b, wk_w, wk_b, wv_w, wv_b, wo_w)
    ]
    nc = _get_nc()
    in_maps = make_in_maps(*args, *wargs)
    res = run_bass_kernel_spmd(
        nc, in_maps, core_ids=list(range(N_CORES)), trace=_trace
    )
    out = np.zeros((SEQ, D_MODEL), np.float32)
    attn = np.empty((1, NUM_HEADS, SEQ, SEQ), np.float32)
    for i in range(N_CORES):
        out += res.results[i]["partial"]
        attn[0, i * HPC : (i + 1) * HPC] = res.results[i]["attn_out"]
    out += np.asarray(wo_b, np.float32)[None, :]
    out = out[None]  # [1, S, D]
    if _trace:
        kernel.last_results = res
    return out, attn


# revision 25
# speedup vs baseline: 1.5127x; 1.0002x over previous
"""Multi-head attention (16 heads, S=4096, D=1024) on 8 TRN2 NeuronCores.

Megatron-style tensor parallelism over heads: core i owns heads (2i, 2i+1).
Each core computes its head slice of the q/k/v projections, full attention
for its 2 heads (writing the softmax probabilities, which are part of the
module output), and a rank-128 partial of the output projection. The host
sums the 8 partials (the "all-reduce") and concatenates the attention
probability slices.

Device algorithm per core (matmuls bf16, softmax/normalization fp32):
  qhT[dh,s] = wq_slice @ q^T  (dh = 128 = 2 heads x 64); same khT, vhT.
  vh[sk,64] per head via PE transpose.
  pass A (per sq-block j of 512, heads row/col-packed on the PE):
    for sk-tile t: LT(h) = khT_t^T-dot-qhT_j  (2 heads as concurrent 64-row
    PE tiles into one [128,1024] PSUM pair) -> exp(LT/tau) (one ACT op)
    -> ctx^T[128,512] += vh^T @ exp  (2 heads as concurrent 64-col PE tiles)
  pass B (per sq-tile jj of 128, independent of pass A):
    logits = qhT_jj^T-dot-khT -> exp(logits/tau) with ACT accum_out giving
    the rowsum -> attn = exp * (1/rowsum)[sq] (DVE) -> DMA
  partial[s,:] = sum_h (ctx_h^T @ woT_h) * (1/rowsum_h)[s]  (PE row-packed)
"""

import sys

sys.path.insert(0, "/opt/trn_rl_repo")

import ml_dtypes
import numpy as np

import concourse.mybir as mybir
from concourse import bacc
from concourse.masks import make_identity
from concourse.tile import TileContext

F32 = mybir.dt.float32
BF16 = mybir.dt.bfloat16
AF = mybir.ActivationFunctionType

D_MODEL = 1024
NUM_HEADS = 16
DEPTH = 64
TAU = 8.0
SEQ = 4096
N_CORES = 8
HPC = NUM_HEADS // N_CORES  # heads per core = 2
DH_SLICE = HPC * DEPTH  # 128 output dims per core


def build_mha_core(seq: int = SEQ, d_model: int = D_MODEL):
    """Build the per-core Bass module (same SPMD program on all 8 cores)."""
    S, D = seq, d_model
    KB = D // 128          # contraction blocks for projections
    NJ = S // 512          # sq blocks (pass A)
    NT = S // 128          # sk tiles (pass A) == sq tiles (pass B)
    NM = S // 512          # sk blocks (pass B)
    SCALE = 1.0 / TAU

    nc = bacc.Bacc("TRN2", target_bir_lowering=False)

    qT = nc.dram_tensor("qT", [D, S], BF16, kind="ExternalInput")
    kT = nc.dram_tensor("kT", [D, S], BF16, kind="ExternalInput")
    vT = nc.dram_tensor("vT", [D, S], BF16, kind="ExternalInput")
    wqT = nc.dram_tensor("wqT", [D, 128], BF16, kind="ExternalInput")
    wkT = nc.dram_tensor("wkT", [D, 128], BF16, kind="ExternalInput")
    wvT = nc.dram_tensor("wvT", [D, 128], BF16, kind="ExternalInput")
    bq = nc.dram_tensor("bq", [128, 1], F32, kind="ExternalInput")
    bk = nc.dram_tensor("bk", [128, 1], F32, kind="ExternalInput")
    bv = nc.dram_tensor("bv", [128, 1], F32, kind="ExternalInput")
    woT = nc.dram_tensor("woT", [128, D], BF16, kind="ExternalInput")
    attn_out = nc.dram_tensor("attn_out", [HPC, S, S], F32, kind="ExternalOutput")
    partial = nc.dram_tensor("partial", [S, D], F32, kind="ExternalOutput")

    with TileContext(nc) as tc:
        with (
            tc.tile_pool(name="consts", bufs=1) as consts,
            tc.tile_pool(name="persist", bufs=1) as persist,
            tc.tile_pool(name="xin", bufs=4) as xin,
            tc.tile_pool(name="expp", bufs=4) as expp,
            tc.tile_pool(name="attnb", bufs=4) as attnb,
            tc.tile_pool(name="outb", bufs=2) as outb,
            tc.tile_pool(name="rsp", bufs=4) as rsp,
            tc.tile_pool(name="ps_a", bufs=2, space="PSUM") as ps_a,
            tc.tile_pool(name="ps_b", bufs=2, space="PSUM") as ps_b,
            tc.tile_pool(name="ps_c", bufs=2, space="PSUM") as ps_c,
        ):
            # ---- constants -------------------------------------------------
            identb = consts.tile([128, 128], BF16)
            make_identity(nc, identb)

            w_sb = {}
            b_sb = {}
            for name, wdram, bdram in (
                ("q", wqT, bq),
                ("k", wkT, bk),
                ("v", wvT, bv),
            ):
                w = consts.tile([128, D], BF16, tag=f"w{name}")
                for kb in range(KB):
                    nc.sync.dma_start(
                        out=w[:, kb * 128 : (kb + 1) * 128],
                        in_=wdram[kb * 128 : (kb + 1) * 128, :],
                    )
                w_sb[name] = w
                b = consts.tile([128, 1], F32, tag=f"b{name}")
                nc.sync.dma_start(out=b, in_=bdram[:, :])
                b_sb[name] = b
            # woT rows for head h at partition base 0: [64, HPC, D]
            woT_sb = consts.tile([64, HPC, D], BF16)
            for h in range(HPC):
                nc.sync.dma_start(
                    out=woT_sb[:, h, :], in_=woT[h * DEPTH : (h + 1) * DEPTH, :]
                )

            # ---- persistent tensors ---------------------------------------
            qhT_sb = persist.tile([128, S], BF16)
            khT_sb = persist.tile([128, S], BF16)
            vhT_sb = persist.tile([128, S], BF16)
            vh_sb = persist.tile([128, HPC, NT, DEPTH], BF16)
            # per-head unnormalized ctx^T
            ctxT_h = [
                persist.tile([64, S], BF16, tag=f"ctxT{h}", name=f"ctxT{h}")
                for h in range(HPC)
            ]
            # 1/rowsum per head as per-sq-partition columns, fp32
            recip_cols = persist.tile([128, HPC, NT], F32)

            # ---- phase 1: projections -> qhT/khT/vhT [128, S] -------------
            for name, xdram, dest in (
                ("k", kT, khT_sb),
                ("q", qT, qhT_sb),
                ("v", vT, vhT_sb),
            ):
                w = w_sb[name]
                for n in range(S // 512):
                    ps = ps_c.tile([128, 512], F32, tag="psc", name="ps")
                    for kb in range(KB):
                        xt = xin.tile([128, 512], BF16, tag="xin")
                        nc.sync.dma_start(
                            out=xt,
                            in_=xdram[
                                kb * 128 : (kb + 1) * 128, n * 512 : (n + 1) * 512
                            ],
                        )
                        nc.tensor.matmul(
                            ps,
                            lhsT=w[:, kb * 128 : (kb + 1) * 128],
                            rhs=xt,
                            start=(kb == 0),
                            stop=(kb == KB - 1),
                        )
                    nc.vector.tensor_scalar_add(
                        dest[:, n * 512 : (n + 1) * 512], ps, b_sb[name]
                    )

            # ---- phase 1.5: vh per head (transposed) ----------------------
            for h in range(HPC):
                hs = slice(h * DEPTH, (h + 1) * DEPTH)
                for t in range(NT):
                    pst = ps_c.tile([128, DEPTH], BF16, tag="psc", name="pst")
                    nc.tensor.transpose(
                        pst,
                        vhT_sb[hs, t * 128 : (t + 1) * 128],
                        identb[hs, hs],
                    )
                    nc.vector.tensor_copy(vh_sb[:, h, t, :], pst)

            # ---- phase 2: attention ---------------------------------------
            ab_tiles = {}

            def emit_passB_unit(h, jj, m):
                hs = slice(h * DEPTH, (h + 1) * DEPTH)
                if m == 0:
                    ab_tiles[h] = attnb.tile(
                        [128, S], F32, tag="attn", name=f"ab{h}"
                    )
                ab = ab_tiles[h]
                pslg = ps_b.tile([128, 512], F32, tag="lg", name="pslg")
                nc.tensor.matmul(
                    pslg,
                    lhsT=qhT_sb[hs, jj * 128 : (jj + 1) * 128],
                    rhs=khT_sb[hs, m * 512 : (m + 1) * 512],
                    start=True,
                    stop=True,
                )
                nc.scalar.activation(
                    ab[:, m * 512 : (m + 1) * 512],
                    pslg,
                    AF.Exp,
                    scale=SCALE,
                )
                if m == NM - 1:
                    rsum = rsp.tile([128, 1], F32, tag="rsum", name="rsum")
                    nc.vector.reduce_sum(rsum, ab, axis=mybir.AxisListType.X)
                    nc.vector.reciprocal(recip_cols[:, h, jj : jj + 1], rsum)
                    nc.vector.tensor_scalar_mul(
                        ab, ab, recip_cols[:, h, jj : jj + 1]
                    )
                    nc.sync.dma_start(
                        out=attn_out[h, jj * 128 : (jj + 1) * 128, :], in_=ab
                    )

            # flat list of pass-B work items, two per pass-A step
            passB = [
                (h, jj, m)
                for jj in range(NT)
                for m in range(NM)
                for h in range(HPC)
            ]
            UB = len(passB)
            TOT = NJ * NT
            done = 0

            for j in range(NJ):
                js = slice(j * 512, (j + 1) * 512)
                psc = [
                    ps_c.tile([64, 512], F32, tag="psc", name=f"psc{h}")
                    for h in range(HPC)
                ]
                prev = None
                for t in range(NT):
                    step = j * NT + t
                    # pass A: QK^T for both heads as concurrent 64-row tiles
                    psl = ps_a.tile([128, 1024], F32, tag="lt", name="psl")
                    for h in range(HPC):
                        hs = slice(h * DEPTH, (h + 1) * DEPTH)
                        nc.tensor.matmul(
                            psl[:, h * 512 : (h + 1) * 512],
                            lhsT=khT_sb[hs, t * 128 : (t + 1) * 128],
                            rhs=qhT_sb[hs, js],
                            start=True,
                            stop=True,
                        )
                    # pass-B filler (independent of fresh exps)
                    lim = (step + 1) * UB // TOT
                    while done < lim:
                        emit_passB_unit(*passB[done])
                        done += 1
                    ex = expp.tile([128, 1024], BF16, tag="exp")
                    nc.scalar.activation(ex, psl, AF.Exp, scale=SCALE)
                    if prev is not None:
                        pex, pt = prev
                        for h in range(HPC):
                            nc.tensor.matmul(
                                psc[h],
                                lhsT=vh_sb[:, h, pt, :],
                                rhs=pex[:, h * 512 : (h + 1) * 512],
                                start=(pt == 0),
                                stop=(pt == NT - 1),
                            )
                    prev = (ex, t)
                pex, pt = prev
                for h in range(HPC):
                    nc.tensor.matmul(
                        psc[h],
                        lhsT=vh_sb[:, h, pt, :],
                        rhs=pex[:, h * 512 : (h + 1) * 512],
                        start=(pt == 0),
                        stop=(pt == NT - 1),
                    )
                for h in range(HPC):
                    nc.vector.tensor_copy(ctxT_h[h][:, js], psc[h])

            # ---- phase 3: output projection (heads as 64-row PE tiles) ----
            for st in range(NT):
                ob = outb.tile([128, D], F32, tag="ob")
                for dhf in range(D // 512):
                    dsl = slice(dhf * 512, (dhf + 1) * 512)
                    pso = []
                    for h in range(HPC):
                        hs = slice(h * DEPTH, (h + 1) * DEPTH)
                        p = ps_a.tile([128, 512], F32, tag="lt", name=f"pso{h}")
                        nc.tensor.matmul(
                            p,
                            lhsT=ctxT_h[h][:, st * 128 : (st + 1) * 128],
                            rhs=woT_sb[:, h, dsl],
                            start=True,
                            stop=True,
                        )
                        pso.append(p)
                    nc.vector.tensor_scalar_mul(
                        ob[:, dsl], pso[0], recip_cols[:, 0, st : st + 1]
                    )
                    nc.vector.scalar_tensor_tensor(
                        out=ob[:, dsl],
                        in0=pso[1],
                        scalar=recip_cols[:, 1, st : st + 1],
                        in1=ob[:, dsl],
                        op0=mybir.AluOpType.mult,
                        op1=mybir.AluOpType.add,
                    )
                nc.sync.dma_start(out=partial[st * 128 : (st + 1) * 128, :], in_=ob)

    nc.compile()
    return nc


def make_in_maps(q, k, v, wq_w, wq_b, wk_w, wk_b, wv_w, wv_b, wo_w):
    bf = ml_dtypes.bfloat16
    qT = np.ascontiguousarray(q.T).astype(bf)
    kT = np.ascontiguousarray(k.T).astype(bf)
    vT = np.ascontiguousarray(v.T).astype(bf)
    in_maps = []
    for i in range(N_CORES):
        sl = slice(i * DH_SLICE, (i + 1) * DH_SLICE)
        in_maps.append(
            {
                "qT": qT,
                "kT": kT,
                "vT": vT,
                "wqT": np.ascontiguousarray(wq_w[sl, :].T).astype(bf),
                "wkT": np.ascontiguousarray(wk_w[sl, :].T).astype(bf),
                "wvT": np.ascontiguousarray(wv_w[sl, :].T).astype(bf),
                "bq": np.ascontiguousarray(wq_b[sl].reshape(-1, 1), dtype=np.float32),
                "bk": np.ascontiguousarray(wk_b[sl].reshape(-1, 1), dtype=np.float32),
                "bv": np.ascontiguousarray(wv_b[sl].reshape(-1, 1), dtype=np.float32),
                "woT": np.ascontiguousarray(wo_w[:, sl].T).astype(bf),
            }
        )
    return in_maps


_NC_CACHE = {}


def _get_nc():
    if "nc" not in _NC_CACHE:
        _NC_CACHE["nc"] = build_mha_core()
    return _NC_CACHE["nc"]


def kernel(
    q,
    k,
    v,
    wq_w,
    wq_b,
    wk_w,
    wk_b,
    wv_w,
    wv_b,
    wo_w,
    wo_b,
    _trace: bool = False,
):
    from concourse.bass_utils import run_bass_kernel_spmd

    args = [np.asarray(x, dtype=np.float32) for x in (q, k, v)]
    wargs = [
        np.asarray(x, dtype=np.float32)
        for x in (wq_w, wq_b, wk_w, wk_b, wv_w, wv_b, wo_w)
    ]
    nc = _get_nc()
    in_maps = make_in_maps(*args, *wargs)
    res = run_bass_kernel_spmd(
        nc, in_maps, core_ids=list(range(N_CORES)), trace=_trace
    )
    out = np.zeros((SEQ, D_MODEL), np.float32)
    attn = np.empty((1, NUM_HEADS, SEQ, SEQ), np.float32)
    for i in range(N_CORES):
        out += res.results[i]["partial"]
        attn[0, i * HPC : (i + 1) * HPC] = res.results[i]["attn_out"]
    out += np.asarray(wo_b, np.float32)[None, :]
    out = out[None]  # [1, S, D]
    if _trace:
        kernel.last_results = res
    return out, attn
